# revision 2
# baseline (speedup 1.0000x reference)
import sys

if "/opt/trn_rl_repo" not in sys.path:
    sys.path.insert(0, "/opt/trn_rl_repo")

import numpy as np

import concourse.bass as bass
import concourse.tile as tile
from concourse import bacc, mybir
from concourse.bass_utils import run_bass_kernel_spmd

T, N, C, A = 32, 64, 512, 32
F1, F2, F3 = 2048, 1024, 512
NC_ = 8          # neuron cores
NB = N // NC_    # batch per core = 8
FREE = NB * T    # 256 free columns, index = n*32 + t
DECAY_SF = None  # set at build time from sigmoid(w_sf0)

_CACHE = {}


def _build(b_out_val: float):
    nc = bacc.Bacc("TRN2", target_bir_lowering=False, debug=False, num_devices=NC_)
    f32 = mybir.dt.float32
    s1T = nc.declare_dram_parameter("s1T", [C, FREE], f32, isOutput=False)
    w1t = nc.declare_dram_parameter("w1t", [C, F1], f32, isOutput=False)
    w2t = nc.declare_dram_parameter("w2t", [F1, F2], f32, isOutput=False)
    w3t = nc.declare_dram_parameter("w3t", [F2, F3], f32, isOutput=False)
    wot = nc.declare_dram_parameter("wot", [F3, 1], f32, isOutput=False)
    mask0 = nc.declare_dram_parameter("mask0", [128, FREE], f32, isOutput=False)
    cmask = nc.declare_dram_parameter("cmask", [1, FREE], f32, isOutput=False)
    out = nc.declare_dram_parameter("out", [1, FREE], f32, isOutput=True)

    AL = mybir.AluOpType

    with tile.TileContext(nc) as tc:
        with (
            tc.tile_pool(name="weights", bufs=1) as wp,
            tc.tile_pool(name="acts", bufs=1) as ap,
            tc.tile_pool(name="psum", bufs=4, space="PSUM") as pp,
        ):
            # ---- load weights / constants ----
            w1 = wp.tile([128, 4, F1], f32)
            w2 = wp.tile([128, 16, F2], f32)
            w3 = wp.tile([128, 8, F3], f32)
            wo = wp.tile([128, 4, 1], f32)
            m0 = wp.tile([128, FREE], f32)
            cm = wp.tile([1, FREE], f32)
            w1r = w1t.ap().rearrange("(kt p) m -> kt p m", p=128)
            w2r = w2t.ap().rearrange("(kt p) m -> kt p m", p=128)
            w3r = w3t.ap().rearrange("(kt p) m -> kt p m", p=128)
            wor = wot.ap().rearrange("(kt p) m -> kt p m", p=128)
            for kt in range(4):
                nc.sync.dma_start(out=w1[:, kt, :], in_=w1r[kt])
            for kt in range(16):
                nc.sync.dma_start(out=w2[:, kt, :], in_=w2r[kt])
            for kt in range(8):
                nc.sync.dma_start(out=w3[:, kt, :], in_=w3r[kt])
            for kt in range(4):
                nc.sync.dma_start(out=wo[:, kt, :], in_=wor[kt])
            nc.sync.dma_start(out=m0[:, :], in_=mask0.ap())
            nc.sync.dma_start(out=cm[:, :], in_=cmask.ap())

            s1 = ap.tile([128, 4, FREE], f32)
            s1r = s1T.ap().rearrange("(kt p) m -> kt p m", p=128)
            for kt in range(4):
                nc.sync.dma_start(out=s1[:, kt, :], in_=s1r[kt])

            def syn_filter(dst, src, ntile):
                # dst[:, j, :] = scan: state = m0*state + src  (per (n) chain,
                # m0 is 0 at t==0 so state restarts per sample)
                for j in range(ntile):
                    nc.vector.tensor_tensor_scan(
                        out=dst[:, j, :], data0=m0[:, :], data1=src[:, j, :],
                        initial=0.0, op0=AL.mult, op1=AL.add,
                    )

            def linear(dst, w, src, kts, mts):
                for mi in range(mts):
                    ps = pp.tile([128, FREE], f32, tag="ps")
                    for kt in range(kts):
                        nc.tensor.matmul(
                            ps[:, :],
                            w[:, kt, bass.ts(mi, 128)],
                            src[:, kt, :],
                            start=(kt == 0), stop=(kt == kts - 1),
                        )
                    nc.scalar.copy(out=dst[:, mi, :], in_=ps[:, :])

            def if_layer(h, ntile):
                # in-place: h becomes the spike train
                v = ap.tile([128, ntile, NB], f32, tag=f"v{ntile}")
                nc.vector.memset(v[:, :, :], 0.0)
                hr = h[:].rearrange("p m (n t) -> p m n t", t=T)
                for t in range(T):
                    ht = hr[:, :, :, t]
                    nc.vector.tensor_tensor(v[:, :, :], v[:, :, :], ht, AL.add)
                    nc.vector.tensor_scalar(ht, v[:, :, :], 1.0, None, AL.is_ge)
                    nc.vector.scalar_tensor_tensor(
                        v[:, :, :], v[:, :, :], 1.0, v[:, :, :], AL.is_lt, AL.mult
                    )

            # block 1: filter -> W1 -> IF
            y1 = ap.tile([128, 4, FREE], f32)
            syn_filter(y1, s1, 4)
            h2 = ap.tile([128, 16, FREE], f32)
            linear(h2, w1, y1, 4, 16)
            if_layer(h2, 16)
            # block 2
            y2 = ap.tile([128, 16, FREE], f32)
            syn_filter(y2, h2, 16)
            h3 = ap.tile([128, 8, FREE], f32)
            linear(h3, w2, y2, 16, 8)
            if_layer(h3, 8)
            # block 3
            y3 = ap.tile([128, 8, FREE], f32)
            syn_filter(y3, h3, 8)
            h4 = ap.tile([128, 4, FREE], f32)
            linear(h4, w3, y3, 8, 4)
            if_layer(h4, 4)
            # head: W_out + b, cumsum over t
            pso = pp.tile([1, FREE], f32, tag="pso")
            for kt in range(4):
                nc.tensor.matmul(
                    pso[:, :], wo[:, kt, :], h4[:, kt, :],
                    start=(kt == 0), stop=(kt == 3),
                )
            pre = ap.tile([1, FREE], f32)
            nc.vector.tensor_scalar_add(pre[:, :], pso[:, :], float(b_out_val))
            acc = ap.tile([1, FREE], f32)
            nc.vector.tensor_tensor_scan(
                out=acc[:, :], data0=cm[:, :], data1=pre[:, :],
                initial=0.0, op0=AL.mult, op1=AL.add,
            )
            nc.sync.dma_start(out=out.ap(), in_=acc[:, :])

    nc.finalize()
    return nc


def _host_front(x, w_jeff, w_cc, w_sf0):
    # transpose (T,N,2,C)->(T,N,C,2); synapse filter tau=2; jeff linear;
    # LIF tau=1.5; synapse filter sigmoid(w_sf0); w_cc contract; IF.
    x = np.asarray(x, np.float32).transpose(0, 1, 3, 2)  # (T,N,C,2)
    f = np.zeros_like(x[0])
    ys = np.empty_like(x)
    for t in range(T):
        f = f * np.float32(0.5) + x[t]
        ys[t] = f
    u = np.einsum("tnci,ai->tnca", ys, np.asarray(w_jeff, np.float32)).astype(np.float32)
    inv_tau = np.float32(1.0 / 1.5)
    v = np.zeros(u.shape[1:], np.float32)
    dec0 = (np.float32(1.0) - np.float32(1.0) / (np.float32(1.0) + np.exp(-np.asarray(w_sf0, np.float32))))
    g = np.zeros(u.shape[1:], np.float32)
    wcc = np.asarray(w_cc, np.float32)[0]  # (A,)
    vI = np.zeros((N, C), np.float32)
    s1 = np.empty((T, N, C), np.float32)
    for t in range(T):
        v = v + (u[t] - v) * inv_tau
        s = (v >= 1.0).astype(np.float32)
        v = v * (1.0 - s)
        g = g * dec0 + s
        z = g @ wcc  # (N,C)
        vI = vI + z
        sI = (vI >= 1.0).astype(np.float32)
        vI = vI * (1.0 - sI)
        s1[t] = sI
    return s1  # (T,N,C)


def kernel(x, w_jeff, w_cc, w_sf0, W1, w_sf1, W2, w_sf2, W3, w_sf3, W_out, b_out):
    x = np.asarray(x, np.float32)
    s1 = _host_front(x, w_jeff, w_cc, w_sf0)  # (T,N,C)

    dec = float(1.0 - 1.0 / (1.0 + np.exp(-float(np.asarray(w_sf1)))))
    tcol = np.arange(FREE) % T
    m0 = np.where(tcol == 0, 0.0, dec).astype(np.float32)
    mask0 = np.tile(m0[None, :], (128, 1))
    cmask = np.where(tcol == 0, 0.0, 1.0).astype(np.float32)[None, :]

    w1t = np.ascontiguousarray(np.asarray(W1, np.float32).T)
    w2t = np.ascontiguousarray(np.asarray(W2, np.float32).T)
    w3t = np.ascontiguousarray(np.asarray(W3, np.float32).T)
    wot = np.ascontiguousarray(np.asarray(W_out, np.float32).T)
    bv = float(np.asarray(b_out).reshape(-1)[0])

    key = ("nc", round(bv, 9))
    if key not in _CACHE:
        _CACHE[key] = _build(bv)
    nc = _CACHE[key]

    in_maps = []
    for c in range(NC_):
        sl = s1[:, c * NB:(c + 1) * NB, :]            # (T, NB, C)
        s1T = np.ascontiguousarray(sl.transpose(2, 1, 0).reshape(C, FREE))
        in_maps.append({
            "s1T": s1T, "w1t": w1t, "w2t": w2t, "w3t": w3t, "wot": wot,
            "mask0": mask0, "cmask": cmask,
        })
    res = run_bass_kernel_spmd(nc, in_maps, core_ids=list(range(NC_)))
    outs = []
    for c in range(NC_):
        o = res.results[c]["out"].reshape(NB, T).T  # (T, NB)
        outs.append(o)
    full = np.concatenate(outs, axis=1)[:, :, None].astype(np.float32)  # (T,N,1)
    return full
